# revision 25
# baseline (speedup 1.0000x reference)
"""Multi-head attention (B=2, L=2048, D=1024, H=16) on 8 TRN2 NeuronCores.

Sharding: batch (2) x head-group (4 heads each) = 8 shards.
Each core computes q/k/v projections for its 4 heads, attention, and a
partial output projection (its 256 rows of Wo); host sums the 4 partials
per batch (fp16 partials, f32 accumulate) and adds bo.

Device dataflow (per core):
  inputs (host-prepped, fp16 except memory_bias):
    qT  [1024, 2048]  = query[b].T          (d on partitions for matmul)
    mT  [1024, 2048]  = memory[b].T
    wq  [1024, 256]   = Wq[:, J] * 0.125    (scale folded)
    wk, wv [1024, 256];  bq*0.125, bk, bv [1, 256]
    wo  [256, 1024]   = Wo[J, :]
    eb  [128, 16]     = exp(memory_bias[b]) col-per-t-chunk (f32)
  phase 1: qT_h [j, f], kT_h [j, t] (head-dim on partitions), v [t, j]
           (natural), v scaled by exp(memory_bias[t]) so the softmax
           bias drops out:
           softmax(s + b) @ v == (exp(s) @ (v * e^b)) / (exp(s) @ e^b)
           DMA issue order is arranged so the tensor engine starts at
           ~7us: (wk, mt0) chunk pairs first, then wv, mt1, wq, qt0, wo.
           fb0's logits interleave with rounds 1-3 (one-round lag).
  phase 2: sT = kT.T@qT -> exp on ScalarE -> AV matmul with an extra
           e^b column producing the softmax denominator row for free ->
           normalize into a [128, f] head-pair tile (denominators of the
           two heads in a pair share one K=2 broadcast matmul) ->
           row-paired output projection partial [f, j] written as fp16.
  AV issue lags exp emission by one t-chunk group so the PE never waits
  on the scalar engine; the drain queue is scanned (not strict FIFO) but
  only the two oldest AV units issue (matches the 2 PSUM accumulators).
"""

import numpy as np

import concourse.bass as bass
import concourse.tile as tile
from concourse import bacc, mybir
from concourse import bass_utils
from concourse.bass import ts, ds

F32 = mybir.dt.float32
F32R = mybir.dt.float32r
BF16 = mybir.dt.bfloat16
FP16 = mybir.dt.float16

B, LQ, LM, D, H = 2, 2048, 2048, 1024, 16
DH = 64
HPC = 4            # heads per core
JC = HPC * DH      # 256 projection cols per core
NCORE = 8
P = 128
FB = 512           # f-block width
NFB = LQ // FB     # 4
NTC = LM // P      # 16 t-chunks
KD = D // P        # 8 contraction chunks for projections
G = 2              # t-chunks per exp group (psum tile [128, G*512])
NG = NTC // G      # 8 groups

VW = DH + 1        # v columns per head incl. denominator column
VS = DH + 2        # v column stride per head (4B alignment in bf16)


def build_kernel(mm_dt=FP16, with_biases=False):
    MM = mm_dt
    nc = bacc.Bacc("TRN2", target_bir_lowering=False, debug=False)

    qTd = nc.dram_tensor("qT", [D, LQ], MM, kind="ExternalInput").ap()
    mTd = nc.dram_tensor("mT", [D, LM], MM, kind="ExternalInput").ap()
    wqd = nc.dram_tensor("wq", [D, JC], MM, kind="ExternalInput").ap()
    wkd = nc.dram_tensor("wk", [D, JC], MM, kind="ExternalInput").ap()
    wvd = nc.dram_tensor("wv", [D, JC], MM, kind="ExternalInput").ap()
    if with_biases:
        bqd = nc.dram_tensor("bq", [1, JC], MM, kind="ExternalInput").ap()
        bkd = nc.dram_tensor("bk", [1, JC], MM, kind="ExternalInput").ap()
        bvd = nc.dram_tensor("bv", [1, JC], MM, kind="ExternalInput").ap()
    wod = nc.dram_tensor("wo", [JC, D], MM, kind="ExternalInput").ap()
    ebd = nc.dram_tensor("eb", [P, NTC], F32, kind="ExternalInput").ap()
    outd = nc.dram_tensor("out", [LQ, D], MM, kind="ExternalOutput").ap()

    def eng(k):
        return nc.sync if k % 2 == 0 else nc.gpsimd

    with tile.TileContext(nc) as tc:
        with (
            tc.tile_pool(name="persist", bufs=1) as persist,
            tc.tile_pool(name="vpool", bufs=1) as vpool,
            tc.tile_pool(name="consts", bufs=1) as consts,
        ):
            # ---- constants ----
            ones_f = consts.tile([1, FB], F32)
            nc.vector.memset(ones_f[:], 1.0)
            ones_row = consts.tile([1, FB], MM)      # rhs for bias matmuls
            nc.vector.tensor_copy(ones_row[:], ones_f[:])
            ones_col = consts.tile([1, P], MM)       # lhsT for v-bias
            nc.vector.tensor_copy(ones_col[:], ones_f[:, 0:P])
            ones4 = consts.tile([P, HPC], F32)       # e^b column fill
            nc.vector.memset(ones4[:], 1.0)
            # h2 selectors for the paired denominator broadcast:
            # sel0 -> partitions 0:64, sel1 -> partitions 64:128
            selF = consts.tile([1, 2 * P], F32)
            nc.vector.memset(selF[:], 0.0)
            nc.vector.memset(selF[0:1, 0:DH], 1.0)
            nc.vector.memset(selF[0:1, P + DH:P + 2 * DH], 1.0)
            selM = consts.tile([1, 2 * P], MM)
            nc.vector.tensor_copy(selM[:], selF[:])
            sel0 = selM[:, 0:P]
            sel1 = selM[:, P:2 * P]
            eb_sb = consts.tile([P, NTC], F32)  # exp(memory_bias), col=tc

            # ---- persistent activations ----
            qTp = [persist.tile([P, LQ], MM, name=f"qTp{i}") for i in range(2)]
            kTp = [persist.tile([P, LM], MM, name=f"kTp{i}") for i in range(2)]
            v_sb = [vpool.tile([P, HPC * VS], MM, name=f"v{t}")
                    for t in range(NTC)]
            wop = [persist.tile([P, D], MM, name=f"wop{i}")
                   for i in range(2)]

            wq_sb = persist.tile([P, KD * JC], MM, name="wq")
            bq_sb = persist.tile([1, JC], MM, name="bq")
            wk_sb = persist.tile([P, KD * JC], MM, name="wk")
            wv_sb = persist.tile([P, KD * JC], MM, name="wv")
            bk_sb = persist.tile([1, JC], MM, name="bk")
            bv_sb = persist.tile([1, JC], MM, name="bv")

            with (
                tc.tile_pool(name="mrhs", bufs=2) as mrhsp,
                tc.tile_pool(name="expp", bufs=34) as expp,
                tc.tile_pool(name="attnp", bufs=6) as attnp,
                tc.tile_pool(name="rp", bufs=3) as rp,
                tc.tile_pool(name="rbp", bufs=4) as rbp,
                tc.tile_pool(name="osb", bufs=4) as osb,
                tc.tile_pool(name="psl", bufs=3, space="PSUM") as psl,
            ):
                exps = {}      # (fb, hp, h2, g) -> [128, G*FB] fp16 tile
                apairs = {}    # (fb, hp) -> [128, FB] attn pair tile
                pend = {}      # (fb, hp) -> finished h2=0 av psum ap
                work_q = []    # ["av", fb, hp, h2, tc_next, av_ap] |
                               # ["op", fb, fc]
                avail = {}     # (fb, hp) -> highest t-chunk issueable

                def kq_proj(w_sb, b_sb, chunks, dstp, col):
                    for hp in range(2):
                        ps = psl.tile([P, G * FB], F32, name="pp", tag="pls")
                        for k in range(KD):
                            nc.tensor.matmul(
                                ps[:, 0:FB],
                                w_sb[:, ds(k * JC + hp * P, P)], chunks[k],
                                start=(k == 0),
                                stop=(not with_biases and k == KD - 1))
                        if with_biases:
                            nc.tensor.matmul(
                                ps[:, 0:FB], b_sb[:, ds(hp * P, P)],
                                ones_row[:], start=False, stop=True)
                        nc.vector.tensor_copy(dstp[hp][:, col], ps[:, 0:FB])

                def q_proj(fb):
                    qt = mrhsp.tile([P, KD * FB], MM, name="qchunk")
                    for k in range(KD):
                        eng(k).dma_start(qt[:, ts(k, FB)],
                                         qTd[ds(k * P, P), ts(fb, FB)])
                    kq_proj(wq_sb, bq_sb,
                            [qt[:, ts(k, FB)] for k in range(KD)],
                            qTp, ts(fb, FB))

                def finish_unit(u):
                    # free the PSUM accumulator fast: stash the (still
                    # unnormalized) attn rows + denominator row, then
                    # normalize this head's rows in place (rb broadcast
                    # read directly from PSUM by the multiply)
                    _, fb, hp, h2, _, av = u
                    if h2 == 0:
                        apairs[(fb, hp)] = attnp.tile([P, FB], MM,
                                                      name="apair")
                    apair = apairs[(fb, hp)]
                    dn = rp.tile([1, FB], F32, name="dn")
                    if fb == NFB - 1 and hp == 1:
                        # tail: scalar engine is idle, vector is critical;
                        # dn first so the reciprocal chain starts sooner
                        nc.scalar.copy(dn[:], av[ds(DH, 1), :])
                        nc.scalar.copy(apair[ds(h2 * DH, DH), :],
                                       av[0:DH, :])
                    else:
                        nc.vector.tensor_copy(
                            apair[ds(h2 * DH, DH), :], av[0:DH, :])
                        nc.vector.tensor_copy(dn[:], av[ds(DH, 1), :])
                    rf = rp.tile([1, FB], F32, name="rf")
                    nc.vector.reciprocal_approx_fast(rf[:], dn[:])
                    rr = rp.tile([1, FB], MM, name="rr")
                    nc.vector.tensor_copy(rr[:], rf[:])
                    rb_ps = psl.tile([P, G * FB], F32, name="rbps", tag="pls")
                    nc.tensor.matmul(rb_ps[ds(h2 * DH, DH), 0:FB],
                                     ones_col[:, 0:DH], rr[:],
                                     start=True, stop=True)
                    nc.vector.tensor_tensor(
                        apair[ds(h2 * DH, DH), :], apair[ds(h2 * DH, DH), :],
                        rb_ps[ds(h2 * DH, DH), 0:FB],
                        op=mybir.AluOpType.mult)
                    if hp == 1 and h2 == 1:
                        for fc in range(4):
                            work_q.append(["op", fb, fc])

                def out_proj_piece(fb, fc):
                    attn = [apairs[(fb, 0)], apairs[(fb, 1)]]
                    o = osb.tile([P, D], MM, name="osb")
                    for jb in range(2):
                        ops = psl.tile([P, G * FB], F32, name="ops",
                                       tag="pls")
                        for hp in range(2):
                            nc.tensor.matmul(
                                ops[:, 0:FB],
                                attn[hp][:, ds(fc * P, P)],
                                wop[hp][:, ts(jb, FB)],
                                start=(hp == 0), stop=(hp == 1))
                        if fb == NFB - 1 and jb == 0:
                            # scalar engine is idle in the tail; split the
                            # copies across scalar + vector
                            nc.scalar.copy(o[:, ts(jb, FB)], ops[:, 0:FB])
                        else:
                            nc.vector.tensor_copy(o[:, ts(jb, FB)],
                                                  ops[:, 0:FB])
                    if fb == NFB - 1 and fc == 3:
                        # last piece: halve across both queues
                        nc.gpsimd.dma_start(
                            outd[ds(fb * FB + fc * P, P), 0:FB], o[:, 0:FB])
                        nc.sync.dma_start(
                            outd[ds(fb * FB + fc * P, P), FB:D], o[:, FB:D])
                    else:
                        de = nc.gpsimd if fc % 2 == 0 else nc.sync
                        de.dma_start(outd[ds(fb * FB + fc * P, P), :], o[:])
                    if fc == 3:
                        apairs.pop((fb, 0))
                        apairs.pop((fb, 1))

                def drain_av(budget):
                    # scan the queue: ops anywhere, but only the two
                    # oldest av units (psx has 2 accumulators)
                    i = 0
                    av_seen = 0
                    while budget > 0 and i < len(work_q):
                        u = work_q[i]
                        if u[0] == "op":
                            if budget < 3:
                                break
                            out_proj_piece(u[1], u[2])
                            work_q.pop(i)
                            budget -= 3
                            continue
                        av_seen += 1
                        if av_seen > 2:
                            # beyond the 2 PSUM accumulators: skip the unit
                            # but keep scanning for runnable op pieces
                            i += 1
                            continue
                        _, fb, hp, h2, _, av = u
                        lim = avail.get((fb, hp), 0)
                        h = 2 * hp + h2
                        while budget > 0 and u[4] < lim:
                            t = u[4]
                            nc.tensor.matmul(
                                av[:], v_sb[t][:, ds(h * VS, VW)],
                                exps[(fb, hp, h2, t // G)][:, ts(t % G, FB)],
                                start=(t == 0), stop=(t == NTC - 1))
                            u[4] += 1
                            budget -= 1
                        if u[4] == NTC:
                            work_q.pop(i)
                            finish_unit(u)
                            continue
                        i += 1

                def logits_step(fb, hp, g, av_budget=6):
                    pls = [psl.tile([P, G * FB], F32, name="pls")
                           for _ in range(2)]
                    for h2 in range(2):
                        for s in range(G):
                            t = g * G + s
                            nc.tensor.matmul(
                                pls[h2][:, ts(s, FB)],
                                kTp[hp][ds(h2 * DH, DH), ts(t, P)],
                                qTp[hp][ds(h2 * DH, DH), ts(fb, FB)],
                                start=True, stop=True)
                        e = expp.tile([P, G * FB], MM, name="exps")
                        nc.scalar.activation(e[:], pls[h2][:],
                                             mybir.ActivationFunctionType.Exp)
                        exps[(fb, hp, h2, g)] = e
                    # two-group lag: AV for group g issues during step g+2,
                    # so the PE never waits on the (trailing) scalar engine
                    avail[(fb, hp)] = max(0, g - 1) * G
                    drain_av(av_budget)

                def logits_substep(fb, hp, t, av_budget):
                    # single t-chunk step (shorter exp latency at the tail)
                    pls = psl.tile([P, G * FB], F32, name="pls")
                    for h2 in range(2):
                        nc.tensor.matmul(
                            pls[:, ts(h2, FB)],
                            kTp[hp][ds(h2 * DH, DH), ts(t, P)],
                            qTp[hp][ds(h2 * DH, DH), ts(fb, FB)],
                            start=True, stop=True)
                        g, s = t // G, t % G
                        if (fb, hp, h2, g) not in exps:
                            exps[(fb, hp, h2, g)] = expp.tile(
                                [P, G * FB], MM, name="exps")
                        nc.scalar.activation(
                            exps[(fb, hp, h2, g)][:, ts(s, FB)],
                            pls[:, ts(h2, FB)],
                            mybir.ActivationFunctionType.Exp)
                    avail[(fb, hp)] = t
                    drain_av(av_budget)

                def enqueue_block(fb, hp):
                    for h2 in range(2):
                        av = psx.tile([P, FB], F32, name="av")[0:VW, :]
                        work_q.append(["av", fb, hp, h2, 0, av])

                # ======= phase 1: k/v/q projections + fb0 logits =======
                with tc.tile_pool(name="ppv", bufs=2, space="PSUM") as ppv:
                    def v_proj_step(chunks, rnd, s):
                        t = rnd * 4 + s
                        psv = ppv.tile([P, JC], F32)
                        for k in range(KD):
                            nc.tensor.matmul(
                                psv[:], chunks[k][:, ds(s * P, P)],
                                wv_sb[:, ts(k, JC)],
                                start=(k == 0),
                                stop=(not with_biases and k == KD - 1))
                        if with_biases:
                            nc.tensor.matmul(
                                psv[:], ones_col[:], bv_sb[:],
                                start=False, stop=True)
                        dst = v_sb[t].rearrange("p (h c) -> p h c", h=HPC)
                        nc.vector.tensor_scalar_mul(
                            dst[:, :, 0:DH],
                            psv[:].rearrange("p (h c) -> p h c", h=HPC),
                            eb_sb[:, ds(t, 1)])
                        nc.vector.tensor_scalar_mul(
                            dst[:, :, ds(DH, 1)],
                            ones4[:].rearrange("p (h o) -> p h o", o=1),
                            eb_sb[:, ds(t, 1)])

                    def mt_round(rnd):
                        mt = mrhsp.tile([P, KD * FB], MM, name="mchunk")
                        for k in range(KD):
                            eng(k).dma_start(mt[:, ts(k, FB)],
                                             mTd[ds(k * P, P), ts(rnd, FB)])
                        return [mt[:, ts(k, FB)] for k in range(KD)]

                    # round 0: (wk_k, mt0_k) DMA pairs so the first
                    # k-projection matmul starts after two transfers
                    mt0 = mrhsp.tile([P, KD * FB], MM, name="mchunk")
                    for k in range(KD):
                        eng(k).dma_start(wk_sb[:, ts(k, JC)],
                                         wkd[ds(k * P, P), :])
                        eng(k).dma_start(mt0[:, ts(k, FB)],
                                         mTd[ds(k * P, P), ts(0, FB)])
                    chunks0 = [mt0[:, ts(k, FB)] for k in range(KD)]
                    if with_biases:
                        nc.gpsimd.dma_start(bk_sb[:], bkd[:])
                        nc.gpsimd.dma_start(bv_sb[:], bvd[:])
                    kq_proj(wk_sb, bk_sb, chunks0, kTp, ts(0, FB))
                    for k in range(KD):
                        eng(k).dma_start(wv_sb[:, ts(k, JC)],
                                         wvd[ds(k * P, P), :])
                    nc.gpsimd.dma_start(eb_sb[:], ebd[:])
                    for s in range(4):
                        v_proj_step(chunks0, 0, s)

                    # round 1: mt1, then wq quarters, qt0, wo
                    chunks1 = mt_round(1)
                    kq_proj(wk_sb, bk_sb, chunks1, kTp, ts(1, FB))
                    for q in range(4):
                        e = eng(q)
                        e.dma_start(
                            wq_sb[:, q * 2 * JC:(q + 1) * 2 * JC].rearrange(
                                "p (k j) -> p k j", k=2),
                            wqd[ds(q * 2 * P, 2 * P), :].rearrange(
                                "(k p) j -> p k j", p=P))
                    if with_biases:
                        nc.gpsimd.dma_start(bq_sb[:], bqd[:])
                    v_proj_step(chunks1, 1, 0)
                    v_proj_step(chunks1, 1, 1)
                    q_proj(0)
                    for i in range(2):
                        eng(i).dma_start(wop[i][:], wod[ds(i * P, P), :])
                    logits_step(0, 0, 0)
                    v_proj_step(chunks1, 1, 2)
                    logits_step(0, 1, 0)
                    v_proj_step(chunks1, 1, 3)
                    logits_step(0, 0, 1)
                    logits_step(0, 1, 1)

                    # rounds 2-3: one-round-lag logits
                    for rnd in (2, 3):
                        chunks = mt_round(rnd)
                        kq_proj(wk_sb, bk_sb, chunks, kTp, ts(rnd, FB))
                        for s in range(4):
                            g = 2 * (rnd - 1) + s // 2
                            logits_step(0, s % 2, g)
                            v_proj_step(chunks, rnd, s)
                    for s in range(4):
                        logits_step(0, s % 2, 6 + s // 2)
                    avail[(0, 0)] = avail[(0, 1)] = NTC

                # ======= steady state: fb blocks with AV in-stream =======
                with tc.tile_pool(name="psx", bufs=2, space="PSUM") as psx:
                    for hp in range(2):
                        enqueue_block(0, hp)
                    q_proj(1)
                    prev = (0, 1)
                    for fb in range(1, NFB):
                        for hp in range(2):
                            enqueue_block(fb, hp)
                            last = fb == NFB - 1 and hp == 1
                            ng = NG - 1 if last else NG
                            for g in range(ng):
                                # finish the previous block with the same
                                # lag so its flush AVs never outrun scalar
                                if g == 1:
                                    avail[prev] = NTC - 2
                                elif g == 2:
                                    avail[prev] = NTC
                                bud = (6, 5, 6, 8)[fb]
                                logits_step(fb, hp, g, av_budget=bud)
                            if last:
                                logits_substep(fb, hp, NTC - 2, 12)
                                logits_substep(fb, hp, NTC - 1, 12)
                                avail[(fb, hp)] = NTC
                            prev = (fb, hp)
                            if hp == 0 and fb < NFB - 1:
                                q_proj(fb + 1)
                    drain_av(10 ** 9)

    nc.compile()
    return nc


_CACHE = {}


def _get_module(with_biases=False):
    key = ("nc", with_biases)
    if key not in _CACHE:
        _CACHE[key] = build_kernel(with_biases=with_biases)
    return _CACHE[key]


def make_in_maps(query, memory, memory_bias, Wq, bq, Wk, bk, Wv, bv, Wo, bo,
                 mm_np=None, with_biases=False):
    if mm_np is None:
        mm_np = np.float16
    query = np.asarray(query, np.float32)
    memory = np.asarray(memory, np.float32)
    memory_bias = np.asarray(memory_bias, np.float32)
    Wq = np.asarray(Wq, np.float32)
    bq = np.asarray(bq, np.float32)
    Wk = np.asarray(Wk, np.float32)
    bk = np.asarray(bk, np.float32)
    Wv = np.asarray(Wv, np.float32)
    bv = np.asarray(bv, np.float32)
    Wo = np.asarray(Wo, np.float32)
    s = np.float32(DH ** -0.5)

    qT = [np.ascontiguousarray(query[b].T).astype(mm_np) for b in range(B)]
    mT = [np.ascontiguousarray(memory[b].T).astype(mm_np) for b in range(B)]
    in_maps = []
    for c in range(NCORE):
        b, g = divmod(c, 4)
        J = slice(g * JC, (g + 1) * JC)
        m = {
            "qT": qT[b],
            "mT": mT[b],
            "wq": (np.ascontiguousarray(Wq[:, J]) * s).astype(mm_np),
            "wk": np.ascontiguousarray(Wk[:, J]).astype(mm_np),
            "wv": np.ascontiguousarray(Wv[:, J]).astype(mm_np),
            "wo": np.ascontiguousarray(Wo[J, :]).astype(mm_np),
            "eb": np.ascontiguousarray(
                np.exp(memory_bias[b].astype(np.float64)).reshape(
                    NTC, P).T).astype(np.float32),
        }
        if with_biases:
            m["bq"] = (bq[J] * s).reshape(1, JC).astype(mm_np)
            m["bk"] = bk[J].reshape(1, JC).astype(mm_np)
            m["bv"] = bv[J].reshape(1, JC).astype(mm_np)
        in_maps.append(m)
    return in_maps


def gather_output(results, bo):
    bo = np.asarray(bo, np.float32)
    out = np.empty((B, LQ, D), np.float32)
    for b in range(B):
        acc = results[4 * b]["out"].astype(np.float32)
        for g in range(1, 4):
            acc = acc + results[4 * b + g]["out"].astype(np.float32)
        out[b] = acc + bo
    return out


def kernel(**inputs):
    wb = any(np.any(np.asarray(inputs[b])) for b in ("bq", "bk", "bv"))
    nc = _get_module(with_biases=wb)
    in_maps = make_in_maps(**inputs, with_biases=wb)
    res = bass_utils.run_bass_kernel_spmd(nc, in_maps,
                                          core_ids=list(range(NCORE)))
    return gather_output(res.results, inputs["bo"])


# revision 31
# speedup vs baseline: 1.0073x; 1.0073x over previous
"""Multi-head attention (B=2, L=2048, D=1024, H=16) on 8 TRN2 NeuronCores.

Sharding: batch (2) x head-group (4 heads each) = 8 shards.
Each core computes q/k/v projections for its 4 heads, attention, and a
partial output projection (its 256 rows of Wo); host sums the 4 partials
per batch (fp16 partials, f32 accumulate) and adds bo.

Device dataflow (per core):
  inputs (host-prepped, fp16 except memory_bias):
    qT  [1024, 2048]  = query[b].T          (d on partitions for matmul)
    mT  [1024, 2048]  = memory[b].T
    wq  [1024, 256]   = Wq[:, J] * 0.125    (scale folded)
    wk, wv [1024, 256];  bq*0.125, bk, bv [1, 256]
    wo  [256, 1024]   = Wo[J, :]
    eb  [128, 16]     = exp(memory_bias[b]) col-per-t-chunk (f32)
  phase 1: qT_h [j, f], kT_h [j, t] (head-dim on partitions), v [t, j]
           (natural), v scaled by exp(memory_bias[t]) so the softmax
           bias drops out:
           softmax(s + b) @ v == (exp(s) @ (v * e^b)) / (exp(s) @ e^b)
           DMA issue order is arranged so the tensor engine starts at
           ~7us: (wk, mt0) chunk pairs first, then wv, mt1, wq, qt0, wo.
           fb0's logits interleave with rounds 1-3 (one-round lag).
  phase 2: sT = kT.T@qT -> exp on ScalarE -> AV matmul with an extra
           e^b column producing the softmax denominator row for free ->
           normalize into a [128, f] head-pair tile (denominators of the
           two heads in a pair share one K=2 broadcast matmul) ->
           row-paired output projection partial [f, j] written as fp16.
  AV issue lags exp emission by one t-chunk group so the PE never waits
  on the scalar engine; the drain queue is scanned (not strict FIFO) but
  only the two oldest AV units issue (matches the 2 PSUM accumulators).
"""

import numpy as np

import concourse.bass as bass
import concourse.tile as tile
from concourse import bacc, mybir
from concourse import bass_utils
from concourse.bass import ts, ds

F32 = mybir.dt.float32
F32R = mybir.dt.float32r
BF16 = mybir.dt.bfloat16
FP16 = mybir.dt.float16

B, LQ, LM, D, H = 2, 2048, 2048, 1024, 16
DH = 64
HPC = 4            # heads per core
JC = HPC * DH      # 256 projection cols per core
NCORE = 8
P = 128
FB = 512           # f-block width
NFB = LQ // FB     # 4
NTC = LM // P      # 16 t-chunks
KD = D // P        # 8 contraction chunks for projections
G = 2              # t-chunks per exp group (psum tile [128, G*512])
NG = NTC // G      # 8 groups

VW = DH + 1        # v columns per head incl. denominator column
VS = DH + 2        # v column stride per head (4B alignment in bf16)


def build_kernel(mm_dt=FP16, with_biases=False):
    MM = mm_dt
    nc = bacc.Bacc("TRN2", target_bir_lowering=False, debug=False)

    qTd = nc.dram_tensor("qT", [D, LQ], MM, kind="ExternalInput").ap()
    mTd = nc.dram_tensor("mT", [D, LM], MM, kind="ExternalInput").ap()
    wqd = nc.dram_tensor("wq", [D, JC], MM, kind="ExternalInput").ap()
    wkd = nc.dram_tensor("wk", [D, JC], MM, kind="ExternalInput").ap()
    wvd = nc.dram_tensor("wv", [D, JC], MM, kind="ExternalInput").ap()
    if with_biases:
        bqd = nc.dram_tensor("bq", [1, JC], MM, kind="ExternalInput").ap()
        bkd = nc.dram_tensor("bk", [1, JC], MM, kind="ExternalInput").ap()
        bvd = nc.dram_tensor("bv", [1, JC], MM, kind="ExternalInput").ap()
    wod = nc.dram_tensor("wo", [JC, D], MM, kind="ExternalInput").ap()
    ebd = nc.dram_tensor("eb", [P, NTC], F32, kind="ExternalInput").ap()
    outd = nc.dram_tensor("out", [LQ, D], MM, kind="ExternalOutput").ap()

    def eng(k):
        return nc.sync if k % 2 == 0 else nc.gpsimd

    with tile.TileContext(nc) as tc:
        with (
            tc.tile_pool(name="persist", bufs=1) as persist,
            tc.tile_pool(name="vpool", bufs=1) as vpool,
            tc.tile_pool(name="consts", bufs=1) as consts,
        ):
            # ---- constants ----
            ones_f = consts.tile([1, FB], F32)
            nc.vector.memset(ones_f[:], 1.0)
            ones_row = consts.tile([1, FB], MM)      # rhs for bias matmuls
            nc.vector.tensor_copy(ones_row[:], ones_f[:])
            ones_col = consts.tile([1, P], MM)       # lhsT for v-bias
            nc.vector.tensor_copy(ones_col[:], ones_f[:, 0:P])
            ones4 = consts.tile([P, HPC], F32)       # e^b column fill
            nc.vector.memset(ones4[:], 1.0)
            # h2 selectors for the paired denominator broadcast:
            # sel0 -> partitions 0:64, sel1 -> partitions 64:128
            selF = consts.tile([1, 2 * P], F32)
            nc.vector.memset(selF[:], 0.0)
            nc.vector.memset(selF[0:1, 0:DH], 1.0)
            nc.vector.memset(selF[0:1, P + DH:P + 2 * DH], 1.0)
            selM = consts.tile([1, 2 * P], MM)
            nc.vector.tensor_copy(selM[:], selF[:])
            sel0 = selM[:, 0:P]
            sel1 = selM[:, P:2 * P]
            eb_sb = consts.tile([P, NTC], F32)  # exp(memory_bias), col=tc

            # ---- persistent activations ----
            qTp = [persist.tile([P, LQ], MM, name=f"qTp{i}") for i in range(2)]
            kTp = [persist.tile([P, LM], MM, name=f"kTp{i}") for i in range(2)]
            v_sb = [vpool.tile([P, HPC * VS], MM, name=f"v{t}")
                    for t in range(NTC)]
            wop = [persist.tile([P, D], MM, name=f"wop{i}")
                   for i in range(2)]

            wq_sb = persist.tile([P, KD * JC], MM, name="wq")
            bq_sb = persist.tile([1, JC], MM, name="bq")
            wk_sb = persist.tile([P, KD * JC], MM, name="wk")
            wv_sb = persist.tile([P, KD * JC], MM, name="wv")
            bk_sb = persist.tile([1, JC], MM, name="bk")
            bv_sb = persist.tile([1, JC], MM, name="bv")

            with (
                tc.tile_pool(name="mrhs", bufs=2) as mrhsp,
                tc.tile_pool(name="expp", bufs=34) as expp,
                tc.tile_pool(name="attnp", bufs=6) as attnp,
                tc.tile_pool(name="rp", bufs=3) as rp,
                tc.tile_pool(name="rbp", bufs=4) as rbp,
                tc.tile_pool(name="osb", bufs=4) as osb,
                tc.tile_pool(name="psl", bufs=3, space="PSUM") as psl,
            ):
                exps = {}      # (fb, hp, h2, g) -> [128, G*FB] fp16 tile
                apairs = {}    # (fb, hp) -> [128, FB] attn pair tile
                pend = {}      # (fb, hp) -> finished h2=0 av psum ap
                work_q = []    # ["av", fb, hp, h2, tc_next, av_ap] |
                               # ["op", fb, fc]
                avail = {}     # (fb, hp) -> highest t-chunk issueable

                def kq_proj(w_sb, b_sb, chunks, dstp, col):
                    for hp in range(2):
                        ps = psl.tile([P, G * FB], F32, name="pp", tag="pls")
                        for k in range(KD):
                            nc.tensor.matmul(
                                ps[:, 0:FB],
                                w_sb[:, ds(k * JC + hp * P, P)], chunks[k],
                                start=(k == 0),
                                stop=(not with_biases and k == KD - 1))
                        if with_biases:
                            nc.tensor.matmul(
                                ps[:, 0:FB], b_sb[:, ds(hp * P, P)],
                                ones_row[:], start=False, stop=True)
                        nc.vector.tensor_copy(dstp[hp][:, col], ps[:, 0:FB])

                def q_proj(fb):
                    qt = mrhsp.tile([P, KD * FB], MM, name="qchunk")
                    for k in range(KD):
                        eng(k).dma_start(qt[:, ts(k, FB)],
                                         qTd[ds(k * P, P), ts(fb, FB)])
                    kq_proj(wq_sb, bq_sb,
                            [qt[:, ts(k, FB)] for k in range(KD)],
                            qTp, ts(fb, FB))

                def finish_unit(u):
                    # free the PSUM accumulator fast: stash the (still
                    # unnormalized) attn rows + denominator row, then
                    # normalize this head's rows in place (rb broadcast
                    # read directly from PSUM by the multiply)
                    _, fb, hp, h2, _, av = u
                    if h2 == 0:
                        apairs[(fb, hp)] = attnp.tile([P, FB], MM,
                                                      name="apair")
                    apair = apairs[(fb, hp)]
                    dn = rp.tile([1, FB], F32, name="dn")
                    if fb == NFB - 1 and hp == 1:
                        # tail: scalar engine is idle, vector is critical;
                        # dn first so the reciprocal chain starts sooner
                        nc.scalar.copy(dn[:], av[ds(DH, 1), :])
                        nc.scalar.copy(apair[ds(h2 * DH, DH), :],
                                       av[0:DH, :])
                    else:
                        nc.vector.tensor_copy(
                            apair[ds(h2 * DH, DH), :], av[0:DH, :])
                        nc.vector.tensor_copy(dn[:], av[ds(DH, 1), :])
                    rf = rp.tile([1, FB], F32, name="rf")
                    nc.vector.reciprocal_approx_fast(rf[:], dn[:])
                    rr = rp.tile([1, FB], MM, name="rr")
                    nc.vector.tensor_copy(rr[:], rf[:])
                    rb_ps = psl.tile([P, G * FB], F32, name="rbps", tag="pls")
                    nc.tensor.matmul(rb_ps[ds(h2 * DH, DH), 0:FB],
                                     ones_col[:, 0:DH], rr[:],
                                     start=True, stop=True)
                    nc.vector.tensor_tensor(
                        apair[ds(h2 * DH, DH), :], apair[ds(h2 * DH, DH), :],
                        rb_ps[ds(h2 * DH, DH), 0:FB],
                        op=mybir.AluOpType.mult)
                    if hp == 1 and h2 == 1:
                        for fc in range(4):
                            work_q.append(["op", fb, fc])

                def out_proj_piece(fb, fc):
                    attn = [apairs[(fb, 0)], apairs[(fb, 1)]]
                    o = osb.tile([P, D], MM, name="osb")
                    for jb in range(2):
                        ops = psl.tile([P, G * FB], F32, name="ops",
                                       tag="pls")
                        for hp in range(2):
                            nc.tensor.matmul(
                                ops[:, 0:FB],
                                attn[hp][:, ds(fc * P, P)],
                                wop[hp][:, ts(jb, FB)],
                                start=(hp == 0), stop=(hp == 1))
                        if fb == NFB - 1 and jb == 0:
                            # scalar engine is idle in the tail; split the
                            # copies across scalar + vector
                            nc.scalar.copy(o[:, ts(jb, FB)], ops[:, 0:FB])
                        else:
                            nc.vector.tensor_copy(o[:, ts(jb, FB)],
                                                  ops[:, 0:FB])
                    if fb == NFB - 1 and fc == 3:
                        # last piece: halve across both queues
                        nc.gpsimd.dma_start(
                            outd[ds(fb * FB + fc * P, P), 0:FB], o[:, 0:FB])
                        nc.sync.dma_start(
                            outd[ds(fb * FB + fc * P, P), FB:D], o[:, FB:D])
                    else:
                        de = nc.gpsimd if fc % 2 == 0 else nc.sync
                        de.dma_start(outd[ds(fb * FB + fc * P, P), :], o[:])
                    if fc == 3:
                        apairs.pop((fb, 0))
                        apairs.pop((fb, 1))

                def drain_av(budget):
                    # scan the queue: ops anywhere, but only the two
                    # oldest av units (psx has 2 accumulators)
                    i = 0
                    av_seen = 0
                    while budget > 0 and i < len(work_q):
                        u = work_q[i]
                        if u[0] == "op":
                            if budget < 3:
                                break
                            out_proj_piece(u[1], u[2])
                            work_q.pop(i)
                            budget -= 3
                            continue
                        av_seen += 1
                        if av_seen > 2:
                            # beyond the 2 PSUM accumulators: skip the unit
                            # but keep scanning for runnable op pieces
                            i += 1
                            continue
                        _, fb, hp, h2, _, av = u
                        lim = avail.get((fb, hp), 0)
                        h = 2 * hp + h2
                        while budget > 0 and u[4] < lim:
                            t = u[4]
                            nc.tensor.matmul(
                                av[:], v_sb[t][:, ds(h * VS, VW)],
                                exps[(fb, hp, h2, t // G)][:, ts(t % G, FB)],
                                start=(t == 0), stop=(t == NTC - 1))
                            u[4] += 1
                            budget -= 1
                        if u[4] == NTC:
                            work_q.pop(i)
                            finish_unit(u)
                            continue
                        i += 1

                def logits_step(fb, hp, g, av_budget=6):
                    pls = [psl.tile([P, G * FB], F32, name="pls")
                           for _ in range(2)]
                    for h2 in range(2):
                        for s in range(G):
                            t = g * G + s
                            nc.tensor.matmul(
                                pls[h2][:, ts(s, FB)],
                                kTp[hp][ds(h2 * DH, DH), ts(t, P)],
                                qTp[hp][ds(h2 * DH, DH), ts(fb, FB)],
                                start=True, stop=True)
                        e = expp.tile([P, G * FB], MM, name="exps")
                        nc.scalar.activation(e[:], pls[h2][:],
                                             mybir.ActivationFunctionType.Exp)
                        exps[(fb, hp, h2, g)] = e
                    # two-group lag: AV for group g issues during step g+2,
                    # so the PE never waits on the (trailing) scalar engine
                    avail[(fb, hp)] = max(0, g - 1) * G
                    drain_av(av_budget)

                def logits_substep(fb, hp, t, av_budget):
                    # single t-chunk step (shorter exp latency at the tail)
                    pls = psl.tile([P, G * FB], F32, name="pls")
                    for h2 in range(2):
                        nc.tensor.matmul(
                            pls[:, ts(h2, FB)],
                            kTp[hp][ds(h2 * DH, DH), ts(t, P)],
                            qTp[hp][ds(h2 * DH, DH), ts(fb, FB)],
                            start=True, stop=True)
                        g, s = t // G, t % G
                        if (fb, hp, h2, g) not in exps:
                            exps[(fb, hp, h2, g)] = expp.tile(
                                [P, G * FB], MM, name="exps")
                        nc.scalar.activation(
                            exps[(fb, hp, h2, g)][:, ts(s, FB)],
                            pls[:, ts(h2, FB)],
                            mybir.ActivationFunctionType.Exp)
                    avail[(fb, hp)] = t
                    drain_av(av_budget)

                def enqueue_block(fb, hp):
                    for h2 in range(2):
                        av = psx.tile([P, FB], F32, name="av")[0:VW, :]
                        work_q.append(["av", fb, hp, h2, 0, av])

                # ======= phase 1: k/v/q projections + fb0 logits =======
                with tc.tile_pool(name="ppv", bufs=2, space="PSUM") as ppv:
                    def v_proj_step(chunks, rnd, s):
                        t = rnd * 4 + s
                        psv = ppv.tile([P, JC], F32)
                        for k in range(KD):
                            nc.tensor.matmul(
                                psv[:], chunks[k][:, ds(s * P, P)],
                                wv_sb[:, ts(k, JC)],
                                start=(k == 0),
                                stop=(not with_biases and k == KD - 1))
                        if with_biases:
                            nc.tensor.matmul(
                                psv[:], ones_col[:], bv_sb[:],
                                start=False, stop=True)
                        dst = v_sb[t].rearrange("p (h c) -> p h c", h=HPC)
                        nc.vector.tensor_scalar_mul(
                            dst[:, :, 0:DH],
                            psv[:].rearrange("p (h c) -> p h c", h=HPC),
                            eb_sb[:, ds(t, 1)])
                        nc.vector.tensor_scalar_mul(
                            dst[:, :, ds(DH, 1)],
                            ones4[:].rearrange("p (h o) -> p h o", o=1),
                            eb_sb[:, ds(t, 1)])

                    def mt_round(rnd):
                        mt = mrhsp.tile([P, KD * FB], MM, name="mchunk")
                        for k in range(KD):
                            eng(k).dma_start(mt[:, ts(k, FB)],
                                             mTd[ds(k * P, P), ts(rnd, FB)])
                        return [mt[:, ts(k, FB)] for k in range(KD)]

                    # round 0: (wk_k, mt0_k) DMA pairs so the first
                    # k-projection matmul starts after two transfers
                    mt0 = mrhsp.tile([P, KD * FB], MM, name="mchunk")
                    for k in range(KD):
                        eng(k).dma_start(wk_sb[:, ts(k, JC)],
                                         wkd[ds(k * P, P), :])
                        eng(k).dma_start(mt0[:, ts(k, FB)],
                                         mTd[ds(k * P, P), ts(0, FB)])
                    chunks0 = [mt0[:, ts(k, FB)] for k in range(KD)]
                    if with_biases:
                        nc.gpsimd.dma_start(bk_sb[:], bkd[:])
                        nc.gpsimd.dma_start(bv_sb[:], bvd[:])
                    kq_proj(wk_sb, bk_sb, chunks0, kTp, ts(0, FB))
                    for k in range(KD):
                        eng(k).dma_start(wv_sb[:, ts(k, JC)],
                                         wvd[ds(k * P, P), :])
                    nc.gpsimd.dma_start(eb_sb[:], ebd[:])
                    for s in range(4):
                        v_proj_step(chunks0, 0, s)

                    # round 1: mt1, then wq quarters, qt0, wo
                    chunks1 = mt_round(1)
                    kq_proj(wk_sb, bk_sb, chunks1, kTp, ts(1, FB))
                    for q in range(4):
                        e = eng(q)
                        e.dma_start(
                            wq_sb[:, q * 2 * JC:(q + 1) * 2 * JC].rearrange(
                                "p (k j) -> p k j", k=2),
                            wqd[ds(q * 2 * P, 2 * P), :].rearrange(
                                "(k p) j -> p k j", p=P))
                    if with_biases:
                        nc.gpsimd.dma_start(bq_sb[:], bqd[:])
                    v_proj_step(chunks1, 1, 0)
                    v_proj_step(chunks1, 1, 1)
                    q_proj(0)
                    for i in range(2):
                        eng(i).dma_start(wop[i][:], wod[ds(i * P, P), :])
                    logits_step(0, 0, 0)
                    v_proj_step(chunks1, 1, 2)
                    logits_step(0, 1, 0)
                    v_proj_step(chunks1, 1, 3)
                    logits_step(0, 0, 1)
                    logits_step(0, 1, 1)

                    # rounds 2-3: one-round-lag logits
                    for rnd in (2, 3):
                        chunks = mt_round(rnd)
                        kq_proj(wk_sb, bk_sb, chunks, kTp, ts(rnd, FB))
                        for s in range(4):
                            g = 2 * (rnd - 1) + s // 2
                            logits_step(0, s % 2, g)
                            v_proj_step(chunks, rnd, s)
                    for s in range(4):
                        logits_step(0, s % 2, 6 + s // 2)
                    avail[(0, 0)] = avail[(0, 1)] = NTC

                # ======= steady state: fb blocks with AV in-stream =======
                with tc.tile_pool(name="psx", bufs=2, space="PSUM") as psx:
                    for hp in range(2):
                        enqueue_block(0, hp)
                    q_proj(1)
                    prev = (0, 1)
                    for fb in range(1, NFB):
                        for hp in range(2):
                            enqueue_block(fb, hp)
                            last = fb == NFB - 1 and hp == 1
                            ng = NG - 1 if last else NG
                            for g in range(ng):
                                # finish the previous block with the same
                                # lag so its flush AVs never outrun scalar
                                if g == 1:
                                    avail[prev] = NTC - 2
                                elif g == 2:
                                    avail[prev] = NTC
                                bud = (6, 6, 5, 8)[fb]
                                logits_step(fb, hp, g, av_budget=bud)
                            if last:
                                logits_substep(fb, hp, NTC - 2, 12)
                                logits_substep(fb, hp, NTC - 1, 12)
                                avail[(fb, hp)] = NTC
                            prev = (fb, hp)
                            if hp == 0 and fb < NFB - 1:
                                q_proj(fb + 1)
                    drain_av(10 ** 9)

    nc.compile()
    return nc


_CACHE = {}


def _get_module(with_biases=False):
    key = ("nc", with_biases)
    if key not in _CACHE:
        _CACHE[key] = build_kernel(with_biases=with_biases)
    return _CACHE[key]


def make_in_maps(query, memory, memory_bias, Wq, bq, Wk, bk, Wv, bv, Wo, bo,
                 mm_np=None, with_biases=False):
    if mm_np is None:
        mm_np = np.float16
    query = np.asarray(query, np.float32)
    memory = np.asarray(memory, np.float32)
    memory_bias = np.asarray(memory_bias, np.float32)
    Wq = np.asarray(Wq, np.float32)
    bq = np.asarray(bq, np.float32)
    Wk = np.asarray(Wk, np.float32)
    bk = np.asarray(bk, np.float32)
    Wv = np.asarray(Wv, np.float32)
    bv = np.asarray(bv, np.float32)
    Wo = np.asarray(Wo, np.float32)
    s = np.float32(DH ** -0.5)

    qT = [np.ascontiguousarray(query[b].T).astype(mm_np) for b in range(B)]
    mT = [np.ascontiguousarray(memory[b].T).astype(mm_np) for b in range(B)]
    in_maps = []
    for c in range(NCORE):
        b, g = divmod(c, 4)
        J = slice(g * JC, (g + 1) * JC)
        m = {
            "qT": qT[b],
            "mT": mT[b],
            "wq": (np.ascontiguousarray(Wq[:, J]) * s).astype(mm_np),
            "wk": np.ascontiguousarray(Wk[:, J]).astype(mm_np),
            "wv": np.ascontiguousarray(Wv[:, J]).astype(mm_np),
            "wo": np.ascontiguousarray(Wo[J, :]).astype(mm_np),
            "eb": np.ascontiguousarray(
                np.exp(memory_bias[b].astype(np.float64)).reshape(
                    NTC, P).T).astype(np.float32),
        }
        if with_biases:
            m["bq"] = (bq[J] * s).reshape(1, JC).astype(mm_np)
            m["bk"] = bk[J].reshape(1, JC).astype(mm_np)
            m["bv"] = bv[J].reshape(1, JC).astype(mm_np)
        in_maps.append(m)
    return in_maps


def gather_output(results, bo):
    bo = np.asarray(bo, np.float32)
    out = np.empty((B, LQ, D), np.float32)
    for b in range(B):
        acc = results[4 * b]["out"].astype(np.float32)
        for g in range(1, 4):
            acc = acc + results[4 * b + g]["out"].astype(np.float32)
        out[b] = acc + bo
    return out


def kernel(**inputs):
    wb = any(np.any(np.asarray(inputs[b])) for b in ("bq", "bk", "bv"))
    nc = _get_module(with_biases=wb)
    in_maps = make_in_maps(**inputs, with_biases=wb)
    res = bass_utils.run_bass_kernel_spmd(nc, in_maps,
                                          core_ids=list(range(NCORE)))
    return gather_output(res.results, inputs["bo"])


# revision 35
# speedup vs baseline: 1.0539x; 1.0463x over previous
"""Multi-head attention (B=2, L=2048, D=1024, H=16) on 8 TRN2 NeuronCores.

Sharding: batch (2) x head-group (4 heads each) = 8 shards.
Each core computes q/k/v projections for its 4 heads, attention, and a
partial output projection (its 256 rows of Wo); host sums the 4 partials
per batch (fp16 partials, f32 accumulate) and adds bo.

Device dataflow (per core):
  inputs (host-prepped, fp16 except memory_bias):
    qT  [1024, 2048]  = query[b].T          (d on partitions for matmul)
    mT  [1024, 2048]  = memory[b].T
    wq  [1024, 256]   = Wq[:, J] * 0.125    (scale folded)
    wk, wv [1024, 256];  bq*0.125, bk, bv [1, 256]
    wo  [256, 1024]   = Wo[J, :]
    eb  [128, 16]     = exp(memory_bias[b]) col-per-t-chunk (f32)
  phase 1: qT_h [j, f], kT_h [j, t] (head-dim on partitions), v [t, j]
           (natural), v scaled by exp(memory_bias[t]) so the softmax
           bias drops out:
           softmax(s + b) @ v == (exp(s) @ (v * e^b)) / (exp(s) @ e^b)
           DMA issue order is arranged so the tensor engine starts at
           ~7us: (wk, mt0) chunk pairs first, then wv, mt1, wq, qt0, wo.
           fb0's logits interleave with rounds 1-3 (one-round lag).
  phase 2: sT = kT.T@qT -> exp on ScalarE -> AV matmul with an extra
           e^b column producing the softmax denominator row for free ->
           normalize into a [128, f] head-pair tile (denominators of the
           two heads in a pair share one K=2 broadcast matmul) ->
           row-paired output projection partial [f, j] written as fp16.
  AV issue lags exp emission by one t-chunk group so the PE never waits
  on the scalar engine; the drain queue is scanned (not strict FIFO) but
  only the two oldest AV units issue (matches the 2 PSUM accumulators).
"""

import numpy as np

import concourse.bass as bass
import concourse.tile as tile
from concourse import bacc, mybir
from concourse import bass_utils
from concourse.bass import ts, ds

F32 = mybir.dt.float32
F32R = mybir.dt.float32r
BF16 = mybir.dt.bfloat16
FP16 = mybir.dt.float16

B, LQ, LM, D, H = 2, 2048, 2048, 1024, 16
DH = 64
HPC = 4            # heads per core
JC = HPC * DH      # 256 projection cols per core
NCORE = 8
P = 128
FB = 512           # f-block width
NFB = LQ // FB     # 4
NTC = LM // P      # 16 t-chunks
KD = D // P        # 8 contraction chunks for projections
G = 2              # t-chunks per exp group (psum tile [128, G*512])
NG = NTC // G      # 8 groups

VW = DH + 1        # v columns per head incl. denominator column
VS = DH + 2        # v column stride per head (4B alignment in bf16)


def build_kernel(mm_dt=FP16, with_biases=False):
    MM = mm_dt
    nc = bacc.Bacc("TRN2", target_bir_lowering=False, debug=False)

    qTd = nc.dram_tensor("qT", [D, LQ], MM, kind="ExternalInput").ap()
    mTd = nc.dram_tensor("mT", [D, LM], MM, kind="ExternalInput").ap()
    wqd = nc.dram_tensor("wq", [D, JC], MM, kind="ExternalInput").ap()
    wkd = nc.dram_tensor("wk", [D, JC], MM, kind="ExternalInput").ap()
    wvd = nc.dram_tensor("wv", [D, JC], MM, kind="ExternalInput").ap()
    if with_biases:
        bqd = nc.dram_tensor("bq", [1, JC], MM, kind="ExternalInput").ap()
        bkd = nc.dram_tensor("bk", [1, JC], MM, kind="ExternalInput").ap()
        bvd = nc.dram_tensor("bv", [1, JC], MM, kind="ExternalInput").ap()
    wod = nc.dram_tensor("wo", [JC, D], MM, kind="ExternalInput").ap()
    ebd = nc.dram_tensor("eb", [P, NTC], F32, kind="ExternalInput").ap()
    outd = nc.dram_tensor("out", [LQ, D], MM, kind="ExternalOutput").ap()

    def eng(k):
        return nc.sync if k % 2 == 0 else nc.gpsimd

    with tile.TileContext(nc) as tc:
        with (
            tc.tile_pool(name="persist", bufs=1) as persist,
            tc.tile_pool(name="vpool", bufs=1) as vpool,
            tc.tile_pool(name="consts", bufs=1) as consts,
        ):
            # ---- constants ----
            ones_f = consts.tile([1, FB], F32)
            nc.vector.memset(ones_f[:], 1.0)
            ones_row = consts.tile([1, FB], MM)      # rhs for bias matmuls
            nc.vector.tensor_copy(ones_row[:], ones_f[:])
            ones_col = consts.tile([1, P], MM)       # lhsT for v-bias
            nc.vector.tensor_copy(ones_col[:], ones_f[:, 0:P])
            ones4 = consts.tile([P, HPC], F32)       # e^b column fill
            nc.vector.memset(ones4[:], 1.0)
            # h2 selectors for the paired denominator broadcast:
            # sel0 -> partitions 0:64, sel1 -> partitions 64:128
            selF = consts.tile([1, 2 * P], F32)
            nc.vector.memset(selF[:], 0.0)
            nc.vector.memset(selF[0:1, 0:DH], 1.0)
            nc.vector.memset(selF[0:1, P + DH:P + 2 * DH], 1.0)
            selM = consts.tile([1, 2 * P], MM)
            nc.vector.tensor_copy(selM[:], selF[:])
            sel0 = selM[:, 0:P]
            sel1 = selM[:, P:2 * P]
            eb_sb = consts.tile([P, NTC], F32)  # exp(memory_bias), col=tc

            # ---- persistent activations ----
            qTp = [persist.tile([P, LQ], MM, name=f"qTp{i}") for i in range(2)]
            kTp = [persist.tile([P, LM], MM, name=f"kTp{i}") for i in range(2)]
            v_sb = [vpool.tile([P, HPC * VS], MM, name=f"v{t}")
                    for t in range(NTC)]
            wop = [persist.tile([P, D], MM, name=f"wop{i}")
                   for i in range(2)]

            wq_sb = persist.tile([P, KD * JC], MM, name="wq")
            bq_sb = persist.tile([1, JC], MM, name="bq")
            wk_sb = persist.tile([P, KD * JC], MM, name="wk")
            wv_sb = persist.tile([P, KD * JC], MM, name="wv")
            bk_sb = persist.tile([1, JC], MM, name="bk")
            bv_sb = persist.tile([1, JC], MM, name="bv")

            with (
                tc.tile_pool(name="mrhs", bufs=2) as mrhsp,
                tc.tile_pool(name="expp", bufs=34) as expp,
                tc.tile_pool(name="attnp", bufs=6) as attnp,
                tc.tile_pool(name="rp", bufs=3) as rp,
                tc.tile_pool(name="rbp", bufs=4) as rbp,
                tc.tile_pool(name="osb", bufs=4) as osb,
                tc.tile_pool(name="psl", bufs=3, space="PSUM") as psl,
            ):
                exps = {}      # (fb, hp, h2, g) -> [128, G*FB] fp16 tile
                apairs = {}    # (fb, hp) -> [128, FB] attn pair tile
                pend = {}      # (fb, hp) -> finished h2=0 av psum ap
                work_q = []    # ["av", fb, hp, h2, tc_next, av_ap] |
                               # ["op", fb, fc]
                avail = {}     # (fb, hp) -> highest t-chunk issueable

                def kq_proj(w_sb, b_sb, chunks, dstp, col):
                    for hp in range(2):
                        ps = psl.tile([P, G * FB], F32, name="pp", tag="pls")
                        for k in range(KD):
                            nc.tensor.matmul(
                                ps[:, 0:FB],
                                w_sb[:, ds(k * JC + hp * P, P)], chunks[k],
                                start=(k == 0),
                                stop=(not with_biases and k == KD - 1))
                        if with_biases:
                            nc.tensor.matmul(
                                ps[:, 0:FB], b_sb[:, ds(hp * P, P)],
                                ones_row[:], start=False, stop=True)
                        nc.vector.tensor_copy(dstp[hp][:, col], ps[:, 0:FB])

                def q_proj(fb):
                    qt = mrhsp.tile([P, KD * FB], MM, name="qchunk")
                    for k in range(KD):
                        eng(k).dma_start(qt[:, ts(k, FB)],
                                         qTd[ds(k * P, P), ts(fb, FB)])
                    kq_proj(wq_sb, bq_sb,
                            [qt[:, ts(k, FB)] for k in range(KD)],
                            qTp, ts(fb, FB))

                def finish_unit(u):
                    # free the PSUM accumulator fast: stash the (still
                    # unnormalized) attn rows + denominator row, then
                    # normalize this head's rows in place (rb broadcast
                    # read directly from PSUM by the multiply)
                    _, fb, hp, h2, _, av = u
                    if h2 == 0:
                        apairs[(fb, hp)] = attnp.tile([P, FB], MM,
                                                      name="apair")
                    apair = apairs[(fb, hp)]
                    dn = rp.tile([1, FB], F32, name="dn")
                    if fb == NFB - 1 and hp == 1:
                        # tail: scalar engine is idle, vector is critical;
                        # dn first so the reciprocal chain starts sooner
                        nc.scalar.copy(dn[:], av[ds(DH, 1), :])
                        nc.scalar.copy(apair[ds(h2 * DH, DH), :],
                                       av[0:DH, :])
                    else:
                        nc.vector.tensor_copy(
                            apair[ds(h2 * DH, DH), :], av[0:DH, :])
                        nc.vector.tensor_copy(dn[:], av[ds(DH, 1), :])
                    rf = rp.tile([1, FB], F32, name="rf")
                    nc.vector.reciprocal_approx_fast(rf[:], dn[:])
                    rr = rp.tile([1, FB], MM, name="rr")
                    nc.vector.tensor_copy(rr[:], rf[:])
                    rb_ps = psl.tile([P, G * FB], F32, name="rbps", tag="pls")
                    nc.tensor.matmul(rb_ps[ds(h2 * DH, DH), 0:FB],
                                     ones_col[:, 0:DH], rr[:],
                                     start=True, stop=True)
                    nc.vector.tensor_tensor(
                        apair[ds(h2 * DH, DH), :], apair[ds(h2 * DH, DH), :],
                        rb_ps[ds(h2 * DH, DH), 0:FB],
                        op=mybir.AluOpType.mult)
                    if hp == 1 and h2 == 1:
                        for fc in range(4):
                            work_q.append(["op", fb, fc])

                def out_proj_piece(fb, fc):
                    attn = [apairs[(fb, 0)], apairs[(fb, 1)]]
                    o = osb.tile([P, D], MM, name="osb")
                    for jb in range(2):
                        ops = psl.tile([P, G * FB], F32, name="ops",
                                       tag="pls")
                        for hp in range(2):
                            nc.tensor.matmul(
                                ops[:, 0:FB],
                                attn[hp][:, ds(fc * P, P)],
                                wop[hp][:, ts(jb, FB)],
                                start=(hp == 0), stop=(hp == 1))
                        if fb == NFB - 1 and jb == 0:
                            # scalar engine is idle in the tail; split the
                            # copies across scalar + vector
                            nc.scalar.copy(o[:, ts(jb, FB)], ops[:, 0:FB])
                        else:
                            nc.vector.tensor_copy(o[:, ts(jb, FB)],
                                                  ops[:, 0:FB])
                    if fb == NFB - 1 and fc == 3:
                        # last piece: halve across both queues
                        nc.gpsimd.dma_start(
                            outd[ds(fb * FB + fc * P, P), 0:FB], o[:, 0:FB])
                        nc.sync.dma_start(
                            outd[ds(fb * FB + fc * P, P), FB:D], o[:, FB:D])
                    else:
                        de = nc.gpsimd if fc % 2 == 0 else nc.sync
                        de.dma_start(outd[ds(fb * FB + fc * P, P), :], o[:])
                    if fc == 3:
                        apairs.pop((fb, 0))
                        apairs.pop((fb, 1))

                def drain_av(budget):
                    # scan the queue: ops anywhere, but only the two
                    # oldest av units (psx has 2 accumulators)
                    i = 0
                    av_seen = 0
                    while budget > 0 and i < len(work_q):
                        u = work_q[i]
                        if u[0] == "op":
                            if budget < 3:
                                break
                            out_proj_piece(u[1], u[2])
                            work_q.pop(i)
                            budget -= 3
                            continue
                        av_seen += 1
                        if av_seen > 2:
                            # beyond the 2 PSUM accumulators: skip the unit
                            # but keep scanning for runnable op pieces
                            i += 1
                            continue
                        _, fb, hp, h2, _, av = u
                        lim = avail.get((fb, hp), 0)
                        h = 2 * hp + h2
                        while budget > 0 and u[4] < lim:
                            t = u[4]
                            nc.tensor.matmul(
                                av[:], v_sb[t][:, ds(h * VS, VW)],
                                exps[(fb, hp, t)][:, ts(h2, FB)],
                                start=(t == 0), stop=(t == NTC - 1))
                            u[4] += 1
                            budget -= 1
                        if u[4] == NTC:
                            work_q.pop(i)
                            finish_unit(u)
                            continue
                        i += 1

                def logits_step(fb, hp, t, av_budget=3, split_act=False):
                    # one t-chunk, both heads of the pair, in ONE psum tile
                    # -> one [128, 1024] exp per step, 3-step pls pipeline
                    pls = psl.tile([P, G * FB], F32, name="pls")
                    for h2 in range(2):
                        nc.tensor.matmul(
                            pls[:, ts(h2, FB)],
                            kTp[hp][ds(h2 * DH, DH), ts(t, P)],
                            qTp[hp][ds(h2 * DH, DH), ts(fb, FB)],
                            start=True, stop=True)
                    e = expp.tile([P, G * FB], MM, name="exps")
                    if split_act:
                        # tail: halve the last exp's latency
                        for h2 in range(2):
                            nc.scalar.activation(
                                e[:, ts(h2, FB)], pls[:, ts(h2, FB)],
                                mybir.ActivationFunctionType.Exp)
                    else:
                        nc.scalar.activation(e[:], pls[:],
                                             mybir.ActivationFunctionType.Exp)
                    exps[(fb, hp, t)] = e
                    # three-chunk lag: AV for chunk t issues at step t+3,
                    # so the PE never waits on the (trailing) scalar engine
                    avail[(fb, hp)] = max(0, t - 2)
                    drain_av(av_budget)

                def enqueue_block(fb, hp):
                    for h2 in range(2):
                        av = psx.tile([P, FB], F32, name="av")[0:VW, :]
                        work_q.append(["av", fb, hp, h2, 0, av])

                # ======= phase 1: k/v/q projections + fb0 logits =======
                with tc.tile_pool(name="ppv", bufs=2, space="PSUM") as ppv:
                    def v_proj_step(chunks, rnd, s):
                        t = rnd * 4 + s
                        psv = ppv.tile([P, JC], F32)
                        for k in range(KD):
                            nc.tensor.matmul(
                                psv[:], chunks[k][:, ds(s * P, P)],
                                wv_sb[:, ts(k, JC)],
                                start=(k == 0),
                                stop=(not with_biases and k == KD - 1))
                        if with_biases:
                            nc.tensor.matmul(
                                psv[:], ones_col[:], bv_sb[:],
                                start=False, stop=True)
                        dst = v_sb[t].rearrange("p (h c) -> p h c", h=HPC)
                        nc.vector.tensor_scalar_mul(
                            dst[:, :, 0:DH],
                            psv[:].rearrange("p (h c) -> p h c", h=HPC),
                            eb_sb[:, ds(t, 1)])
                        nc.vector.tensor_scalar_mul(
                            dst[:, :, ds(DH, 1)],
                            ones4[:].rearrange("p (h o) -> p h o", o=1),
                            eb_sb[:, ds(t, 1)])

                    def mt_round(rnd):
                        mt = mrhsp.tile([P, KD * FB], MM, name="mchunk")
                        for k in range(KD):
                            eng(k).dma_start(mt[:, ts(k, FB)],
                                             mTd[ds(k * P, P), ts(rnd, FB)])
                        return [mt[:, ts(k, FB)] for k in range(KD)]

                    # round 0: (wk_k, mt0_k) DMA pairs so the first
                    # k-projection matmul starts after two transfers
                    mt0 = mrhsp.tile([P, KD * FB], MM, name="mchunk")
                    for k in range(KD):
                        eng(k).dma_start(wk_sb[:, ts(k, JC)],
                                         wkd[ds(k * P, P), :])
                        eng(k).dma_start(mt0[:, ts(k, FB)],
                                         mTd[ds(k * P, P), ts(0, FB)])
                    chunks0 = [mt0[:, ts(k, FB)] for k in range(KD)]
                    if with_biases:
                        nc.gpsimd.dma_start(bk_sb[:], bkd[:])
                        nc.gpsimd.dma_start(bv_sb[:], bvd[:])
                    kq_proj(wk_sb, bk_sb, chunks0, kTp, ts(0, FB))
                    for k in range(KD):
                        eng(k).dma_start(wv_sb[:, ts(k, JC)],
                                         wvd[ds(k * P, P), :])
                    nc.gpsimd.dma_start(eb_sb[:], ebd[:])
                    for s in range(4):
                        v_proj_step(chunks0, 0, s)

                    # round 1: mt1, then wq quarters, qt0, wo
                    chunks1 = mt_round(1)
                    kq_proj(wk_sb, bk_sb, chunks1, kTp, ts(1, FB))
                    for q in range(4):
                        e = eng(q)
                        e.dma_start(
                            wq_sb[:, q * 2 * JC:(q + 1) * 2 * JC].rearrange(
                                "p (k j) -> p k j", k=2),
                            wqd[ds(q * 2 * P, 2 * P), :].rearrange(
                                "(k p) j -> p k j", p=P))
                    if with_biases:
                        nc.gpsimd.dma_start(bq_sb[:], bqd[:])
                    v_proj_step(chunks1, 1, 0)
                    v_proj_step(chunks1, 1, 1)
                    q_proj(0)
                    for i in range(2):
                        eng(i).dma_start(wop[i][:], wod[ds(i * P, P), :])
                    logits_step(0, 0, 0)
                    logits_step(0, 0, 1)
                    v_proj_step(chunks1, 1, 2)
                    logits_step(0, 1, 0)
                    logits_step(0, 1, 1)
                    v_proj_step(chunks1, 1, 3)
                    logits_step(0, 0, 2)
                    logits_step(0, 0, 3)
                    logits_step(0, 1, 2)
                    logits_step(0, 1, 3)

                    # rounds 2-3: one-round-lag logits
                    for rnd in (2, 3):
                        chunks = mt_round(rnd)
                        kq_proj(wk_sb, bk_sb, chunks, kTp, ts(rnd, FB))
                        for s in range(4):
                            t0 = 4 * (rnd - 1) + 2 * (s // 2)
                            logits_step(0, s % 2, t0)
                            logits_step(0, s % 2, t0 + 1)
                            v_proj_step(chunks, rnd, s)
                    for s in range(4):
                        t0 = 12 + 2 * (s // 2)
                        logits_step(0, s % 2, t0)
                        logits_step(0, s % 2, t0 + 1)
                    avail[(0, 0)] = avail[(0, 1)] = NTC

                # ======= steady state: fb blocks with AV in-stream =======
                with tc.tile_pool(name="psx", bufs=2, space="PSUM") as psx:
                    for hp in range(2):
                        enqueue_block(0, hp)
                    q_proj(1)
                    prev = (0, 1)
                    for fb in range(1, NFB):
                        for hp in range(2):
                            enqueue_block(fb, hp)
                            last = fb == NFB - 1 and hp == 1
                            for t in range(NTC):
                                # finish the previous block with the same
                                # lag so its flush AVs never outrun scalar
                                if t == 2:
                                    avail[prev] = NTC - 2
                                elif t == 4:
                                    avail[prev] = NTC
                                bud = (3, 3, 3, 4)[fb]
                                logits_step(fb, hp, t, av_budget=bud,
                                            split_act=(last and t == NTC - 1))
                            if last:
                                avail[(fb, hp)] = NTC
                            prev = (fb, hp)
                            if hp == 0 and fb < NFB - 1:
                                q_proj(fb + 1)
                    drain_av(10 ** 9)

    nc.compile()
    return nc


_CACHE = {}


def _get_module(with_biases=False):
    key = ("nc", with_biases)
    if key not in _CACHE:
        _CACHE[key] = build_kernel(with_biases=with_biases)
    return _CACHE[key]


def make_in_maps(query, memory, memory_bias, Wq, bq, Wk, bk, Wv, bv, Wo, bo,
                 mm_np=None, with_biases=False):
    if mm_np is None:
        mm_np = np.float16
    query = np.asarray(query, np.float32)
    memory = np.asarray(memory, np.float32)
    memory_bias = np.asarray(memory_bias, np.float32)
    Wq = np.asarray(Wq, np.float32)
    bq = np.asarray(bq, np.float32)
    Wk = np.asarray(Wk, np.float32)
    bk = np.asarray(bk, np.float32)
    Wv = np.asarray(Wv, np.float32)
    bv = np.asarray(bv, np.float32)
    Wo = np.asarray(Wo, np.float32)
    s = np.float32(DH ** -0.5)

    qT = [np.ascontiguousarray(query[b].T).astype(mm_np) for b in range(B)]
    mT = [np.ascontiguousarray(memory[b].T).astype(mm_np) for b in range(B)]
    in_maps = []
    for c in range(NCORE):
        b, g = divmod(c, 4)
        J = slice(g * JC, (g + 1) * JC)
        m = {
            "qT": qT[b],
            "mT": mT[b],
            "wq": (np.ascontiguousarray(Wq[:, J]) * s).astype(mm_np),
            "wk": np.ascontiguousarray(Wk[:, J]).astype(mm_np),
            "wv": np.ascontiguousarray(Wv[:, J]).astype(mm_np),
            "wo": np.ascontiguousarray(Wo[J, :]).astype(mm_np),
            "eb": np.ascontiguousarray(
                np.exp(memory_bias[b].astype(np.float64)).reshape(
                    NTC, P).T).astype(np.float32),
        }
        if with_biases:
            m["bq"] = (bq[J] * s).reshape(1, JC).astype(mm_np)
            m["bk"] = bk[J].reshape(1, JC).astype(mm_np)
            m["bv"] = bv[J].reshape(1, JC).astype(mm_np)
        in_maps.append(m)
    return in_maps


def gather_output(results, bo):
    bo = np.asarray(bo, np.float32)
    out = np.empty((B, LQ, D), np.float32)
    for b in range(B):
        acc = results[4 * b]["out"].astype(np.float32)
        for g in range(1, 4):
            acc = acc + results[4 * b + g]["out"].astype(np.float32)
        out[b] = acc + bo
    return out


def kernel(**inputs):
    wb = any(np.any(np.asarray(inputs[b])) for b in ("bq", "bk", "bv"))
    nc = _get_module(with_biases=wb)
    in_maps = make_in_maps(**inputs, with_biases=wb)
    res = bass_utils.run_bass_kernel_spmd(nc, in_maps,
                                          core_ids=list(range(NCORE)))
    return gather_output(res.results, inputs["bo"])
